# revision 12
# baseline (speedup 1.0000x reference)
"""Multi-head attention kernel for Trainium2, sharded over 8 NeuronCores.

Problem: B=2, S=2048, HIDDEN=1024, 16 heads, head_dim=64, fp32.

Sharding (data + tensor parallel per the hint): core c handles batch b=c//4
and head-group g=c%4 (4 heads = 256 hidden columns). QKV projections are
column-sharded, output projection row-sharded; each core returns a partial
out^T [1024, 2048] and the host sums the 4 partials per batch (the
row-parallel all-reduce) and transposes.

Per-core device program (all matmul layouts chosen to avoid transposes of
big intermediates; only x is transposed once via PE matmul-vs-identity):
  x^T[k,s]   = PE transpose of x                              (8 MB, once)
  Q^T/K^T    = Wq/Wk_cols^T @ x^T     -> [256, 2048] in SBUF  (head-major)
  V          = x @ Wv_cols            -> [2048, 256] natural, stored per
               kpos-tile with a ones column appended per head ([128, 4*65])
  scores^T   = K_h^T.T @ Q_h^T        -> [kpos 128, q 512] per tile; the two
               heads of a 128-partition pair run row-packed (K=64 each,
               tile_position (0,0)/(64,0)) into two PSUM banks
  P^T        = exp(SCALE*scores^T + negmask[kpos])   (mask rides the ACT
               per-partition bias; scale is the free ACT affine)
  ctx~^T     = [V_h | 1]^T @ P^T accumulated over kpos in PSUM -> [65, q];
               row 64 is the softmax denominator for free
  ctx^T     /= denom (reciprocal + PE broadcast via selector matmul)
  out^T     += Wo_rows^T @ ctx^T + bo_eff            (row-parallel partial)

bv is folded into bo_eff on the host (bv @ Wo_rows), bo added on one core
per batch group.
"""

import sys
import types

import numpy as np

import concourse.bass as bass
import concourse.tile as tile
from concourse import bacc, mybir
from concourse.bass_utils import run_bass_kernel_spmd
from concourse.masks import make_identity


def _install_ntff_hook_shim():
    """The agent image's antenv lacks axon_hooks, so trace=True dies on
    import. Recreate the module with the boot script's ctypes-based hook
    so NTFF profiling works."""
    if "antenv.axon_hooks" in sys.modules:
        return
    mod = types.ModuleType("antenv.axon_hooks")
    mod._hook = None

    def set_axon_ntff_profile_hook(h):
        mod._hook = h

    def get_axon_ntff_profile_hook():
        return mod._hook

    mod.set_axon_ntff_profile_hook = set_axon_ntff_profile_hook
    mod.get_axon_ntff_profile_hook = get_axon_ntff_profile_hook
    sys.modules["antenv.axon_hooks"] = mod
    try:
        from trn_agent_boot.trn_boot import _ntff_profile_via_ctypes

        mod._hook = _ntff_profile_via_ctypes("/opt/axon/libaxon_pjrt.so")
    except Exception:
        mod._hook = None


_install_ntff_hook_shim()

F32 = mybir.dt.float32

B = 2
S = 2048
HID = 1024
NH = 16  # total heads
DH = 64  # head dim
NCORES = 8
GROUPS = 4  # head groups (cores per batch)
NHC = 256  # hidden columns per core (4 heads * 64)
KT = 16  # kpos tiles of 128
SCALE = DH**-0.5

AF = mybir.ActivationFunctionType


def build_program():
    nc = bacc.Bacc(
        "TRN2",
        target_bir_lowering=False,
        debug=False,
        enable_asserts=False,
        num_devices=NCORES,
    )

    x_d = nc.dram_tensor("x", [S, HID], F32, kind="ExternalInput")
    wq_d = nc.dram_tensor("wq", [HID, NHC], F32, kind="ExternalInput")
    wk_d = nc.dram_tensor("wk", [HID, NHC], F32, kind="ExternalInput")
    wv_d = nc.dram_tensor("wv", [HID, NHC], F32, kind="ExternalInput")
    wo_d = nc.dram_tensor("wo", [NHC, HID], F32, kind="ExternalInput")
    bq_d = nc.dram_tensor("bq", [128, 2], F32, kind="ExternalInput")
    bk_d = nc.dram_tensor("bk", [128, 2], F32, kind="ExternalInput")
    bo_d = nc.dram_tensor("bo", [128, 8], F32, kind="ExternalInput")
    nm_d = nc.dram_tensor("negmask", [128, KT], F32, kind="ExternalInput")
    sel_d = nc.dram_tensor("sel", [128, 2, 128], F32, kind="ExternalInput")
    out_d = nc.dram_tensor("outT", [HID, S], F32, kind="ExternalOutput")

    with tile.TileContext(nc) as tc:
        with tc.tile_pool(name="persist", bufs=1) as persist:
            ident = persist.tile([128, 128], F32, tag="ident")
            make_identity(nc, ident)
            negm = persist.tile([128, KT], F32, tag="negm")
            nc.sync.dma_start(out=negm, in_=nm_d[:])
            wo_sb = persist.tile([128, 2, HID], F32, tag="wo")
            nc.sync.dma_start(
                out=wo_sb, in_=wo_d[:].rearrange("(a p) n -> p a n", p=128)
            )
            bq_sb = persist.tile([128, 2], F32, tag="bq")
            nc.sync.dma_start(out=bq_sb, in_=bq_d[:])
            bk_sb = persist.tile([128, 2], F32, tag="bk")
            nc.sync.dma_start(out=bk_sb, in_=bk_d[:])
            bo_sb = persist.tile([128, 8], F32, tag="bo")
            nc.sync.dma_start(out=bo_sb, in_=bo_d[:])

            # Q^T / K^T head-pair tiles: partition = (head-in-pair, d),
            # free = (pair, q)
            qt = persist.tile([128, 2, S], F32, tag="qt")
            kt_sb = persist.tile([128, 2, S], F32, tag="kt")
            # V per kpos-tile, per head 64 cols + ones col: [128, KT, 4*65]
            vall = persist.tile([128, KT, 4 * 65], F32, tag="vall")
            # ctx^T (same layout as qt), normalized in place later
            ctx_sb = persist.tile([128, 2, S], F32, tag="ctx")
            # softmax denominators / reciprocals: head h lives on partition
            # 32*h (engine APs must start at a 32-partition boundary); unused
            # partitions are primed with 1.0 so the reciprocal stays finite.
            den = persist.tile([128, S], F32, tag="den")
            rec = persist.tile([128, S], F32, tag="rec")
            scr = persist.tile([128, S], F32, tag="scr")
            nc.vector.memset(den, 1.0)
            # selector for broadcasting recip rows across partitions
            sel = persist.tile([128, 2, 128], F32, tag="sel")
            nc.sync.dma_start(out=sel, in_=sel_d[:])

            # ---------------- Phase 1: x^T and QKV projections ----------------
            with tc.tile_pool(name="xnat", bufs=2) as xnat_p, \
                 tc.tile_pool(name="win", bufs=1) as win_p, \
                 tc.tile_pool(name="xtp", bufs=1) as xtp, \
                 tc.tile_pool(name="ps_t", bufs=2, space="PSUM") as ps_t, \
                 tc.tile_pool(name="ps_p", bufs=2, space="PSUM") as ps_p, \
                 tc.tile_pool(name="ps_v", bufs=2, space="PSUM") as ps_v:
                wq_sb = win_p.tile([128, 8, NHC], F32, tag="wq")
                nc.sync.dma_start(
                    out=wq_sb, in_=wq_d[:].rearrange("(a p) n -> p a n", p=128)
                )
                wk_sb = win_p.tile([128, 8, NHC], F32, tag="wk")
                nc.sync.dma_start(
                    out=wk_sb, in_=wk_d[:].rearrange("(a p) n -> p a n", p=128)
                )
                wv_sb = win_p.tile([128, 8, NHC], F32, tag="wv")
                nc.sync.dma_start(
                    out=wv_sb, in_=wv_d[:].rearrange("(a p) n -> p a n", p=128)
                )

                # x^T in SBUF: [k-in-tile 128, (ktile 8, s 2048)]
                xT = xtp.tile([128, 8, S], F32, tag="xT")
                x_re = x_d[:].rearrange("(c m p) k -> c p m k", p=128, m=2)
                for chunk in range(8):
                    xn = xnat_p.tile([128, 2, HID], F32, tag="xn")
                    nc.sync.dma_start(out=xn, in_=x_re[chunk])
                    for m in range(2):
                        s0 = chunk * 256 + m * 128
                        for kg in range(2):
                            tps = ps_t.tile([128, 4, 128], F32, tag="t")
                            for kk in range(4):
                                kj = kg * 4 + kk
                                nc.tensor.matmul(
                                    tps[:, kk, :],
                                    lhsT=xn[:, m, kj * 128 : (kj + 1) * 128],
                                    rhs=ident,
                                    start=True,
                                    stop=True,
                                )
                            nc.vector.tensor_copy(
                                xT[:, kg * 4 : (kg + 1) * 4, s0 : s0 + 128], tps
                            )

                # Q^T, K^T: lhsT = W cols, rhs = x^T
                for wsb, bsb, dst in ((wq_sb, bq_sb, qt), (wk_sb, bk_sb, kt_sb)):
                    for pt in range(2):
                        for j4 in range(4):
                            pp = ps_p.tile([128, 512], F32, tag="p")
                            for kj in range(8):
                                nc.tensor.matmul(
                                    pp,
                                    lhsT=wsb[:, kj, pt * 128 : (pt + 1) * 128],
                                    rhs=xT[:, kj, j4 * 512 : (j4 + 1) * 512],
                                    start=(kj == 0),
                                    stop=(kj == 7),
                                )
                            nc.vector.tensor_scalar_add(
                                dst[:, pt, j4 * 512 : (j4 + 1) * 512],
                                pp,
                                bsb[:, pt : pt + 1],
                            )

                # V natural: lhsT = x^T s-tiles, rhs = Wv
                for mt in range(KT):
                    pv = ps_v.tile([128, NHC], F32, tag="v")
                    for kj in range(8):
                        nc.tensor.matmul(
                            pv,
                            lhsT=xT[:, kj, mt * 128 : (mt + 1) * 128],
                            rhs=wv_sb[:, kj, :],
                            start=(kj == 0),
                            stop=(kj == 7),
                        )
                    v_slot = vall[:, mt, :].rearrange("p (h e) -> p h e", h=4)
                    nc.vector.tensor_copy(
                        v_slot[:, :, 0:64], pv.rearrange("p (h d) -> p h d", h=4)
                    )
                    nc.gpsimd.memset(v_slot[:, :, 64:65], 1.0)

            # ---------------- Phase 2: attention ----------------
            with tc.tile_pool(name="ps_s", bufs=2, space="PSUM") as ps_s, \
                 tc.tile_pool(name="ps_c", bufs=2, space="PSUM") as ps_c, \
                 tc.tile_pool(name="expp", bufs=3) as expp:
                for pt in range(2):
                    for j4 in range(4):
                        q0 = j4 * 512
                        cps = [
                            ps_c.tile([65, 512], F32, tag=f"c{h}", name=f"cps{h}")
                            for h in range(2)
                        ]
                        for t in range(KT):
                            sps = ps_s.tile([128, 2, 512], F32, tag="s")
                            for hh in range(2):
                                nc.tensor.matmul(
                                    sps[:, hh, :],
                                    lhsT=kt_sb[
                                        hh * 64 : (hh + 1) * 64,
                                        pt,
                                        t * 128 : (t + 1) * 128,
                                    ],
                                    rhs=qt[
                                        hh * 64 : (hh + 1) * 64, pt, q0 : q0 + 512
                                    ],
                                    start=True,
                                    stop=True,
                                    tile_position=(hh * 64, 0),
                                )
                            ep = expp.tile([128, 2, 512], F32, tag="e")
                            nc.scalar.activation(
                                ep,
                                sps,
                                AF.Exp,
                                bias=negm[:, t : t + 1],
                                scale=float(SCALE),
                            )
                            for hh in range(2):
                                c0 = (2 * pt + hh) * 65
                                nc.tensor.matmul(
                                    cps[hh],
                                    lhsT=vall[:, t, c0 : c0 + 65],
                                    rhs=ep[:, hh, :],
                                    start=(t == 0),
                                    stop=(t == KT - 1),
                                )
                        for hh in range(2):
                            nc.vector.tensor_copy(
                                ctx_sb[hh * 64 : (hh + 1) * 64, pt, q0 : q0 + 512],
                                cps[hh][0:64, :],
                            )
                            h = 2 * pt + hh
                            nc.vector.tensor_copy(
                                den[32 * h : 32 * h + 1, q0 : q0 + 512],
                                cps[hh][64:65, :],
                            )

            # ---------------- Phase 3: normalize + output projection ----------
            nc.vector.reciprocal_approx_accurate(out=rec, in_=den, scratch=scr)
            with tc.tile_pool(name="ps_b", bufs=2, space="PSUM") as ps_b, \
                 tc.tile_pool(name="ps_o", bufs=2, space="PSUM") as ps_o, \
                 tc.tile_pool(name="outp", bufs=3) as outp:
                for pt in range(2):
                    for j4 in range(4):
                        q0 = j4 * 512
                        bc = ps_b.tile([128, 512], F32, tag="b")
                        nc.tensor.matmul(
                            bc,
                            lhsT=sel[:, pt, :],
                            rhs=rec[:, q0 : q0 + 512],
                            start=True,
                            stop=True,
                        )
                        nc.vector.tensor_mul(
                            ctx_sb[:, pt, q0 : q0 + 512],
                            ctx_sb[:, pt, q0 : q0 + 512],
                            bc,
                        )
                out_re = out_d[:].rearrange("(a p) s -> a p s", p=128)
                for mt in range(8):
                    ot = outp.tile([128, S], F32, tag="o")
                    for j4 in range(4):
                        q0 = j4 * 512
                        po = ps_o.tile([128, 512], F32, tag="po")
                        for pt in range(2):
                            nc.tensor.matmul(
                                po,
                                lhsT=wo_sb[:, pt, mt * 128 : (mt + 1) * 128],
                                rhs=ctx_sb[:, pt, q0 : q0 + 512],
                                start=(pt == 0),
                                stop=(pt == 1),
                            )
                        # split PSUM->SBUF copies between DVE and ACT
                        if j4 % 2 == 0:
                            nc.vector.tensor_scalar_add(
                                ot[:, q0 : q0 + 512], po, bo_sb[:, mt : mt + 1]
                            )
                        else:
                            nc.scalar.add(
                                ot[:, q0 : q0 + 512], po, bo_sb[:, mt : mt + 1]
                            )
                    nc.sync.dma_start(out=out_re[mt], in_=ot)

    nc.compile()
    return nc


_PROGRAM = None


def _get_program():
    global _PROGRAM
    if _PROGRAM is None:
        _PROGRAM = build_program()
    return _PROGRAM


def make_in_maps(inputs):
    hs = np.asarray(inputs["hidden_states"], dtype=np.float32)
    mask = np.asarray(inputs["attention_mask"], dtype=np.float32)
    Wq = np.asarray(inputs["Wq"], dtype=np.float32)
    bq = np.asarray(inputs["bq"], dtype=np.float32)
    Wk = np.asarray(inputs["Wk"], dtype=np.float32)
    bk = np.asarray(inputs["bk"], dtype=np.float32)
    Wv = np.asarray(inputs["Wv"], dtype=np.float32)
    bv = np.asarray(inputs["bv"], dtype=np.float32)
    Wo = np.asarray(inputs["Wo"], dtype=np.float32)
    bo = np.asarray(inputs["bo"], dtype=np.float32)

    # selector: sel[k, pt, m] = 1 iff k == 32*(2*pt + m//64) (same on all
    # cores; head h's reciprocal lives on partition 32*h)
    sel = np.zeros((128, 2, 128), np.float32)
    for pt in range(2):
        for hh in range(2):
            sel[32 * (2 * pt + hh), pt, hh * 64 : (hh + 1) * 64] = 1.0

    in_maps = []
    for c in range(NCORES):
        b = c // GROUPS
        g = c % GROUPS
        cols = slice(g * NHC, (g + 1) * NHC)
        negm = ((1.0 - mask[b]) * -10000.0).astype(np.float32)
        # bv folded through the output projection; bo added on one core/batch
        bo_eff = bv[cols] @ Wo[cols, :]
        if g == 0:
            bo_eff = bo_eff + bo
        in_maps.append(
            {
                "x": np.ascontiguousarray(hs[b]),
                "wq": np.ascontiguousarray(Wq[:, cols]),
                "wk": np.ascontiguousarray(Wk[:, cols]),
                "wv": np.ascontiguousarray(Wv[:, cols]),
                "wo": np.ascontiguousarray(Wo[cols, :]),
                "bq": np.ascontiguousarray(bq[cols].reshape(2, 128).T),
                "bk": np.ascontiguousarray(bk[cols].reshape(2, 128).T),
                "bo": np.ascontiguousarray(
                    bo_eff.astype(np.float32).reshape(8, 128).T
                ),
                "negmask": np.ascontiguousarray(negm.reshape(KT, 128).T),
                "sel": sel,
            }
        )
    return in_maps


def gather_output(per_core_outs):
    out = np.empty((B, S, HID), dtype=np.float32)
    for b in range(B):
        acc = per_core_outs[b * GROUPS]["outT"].astype(np.float32)
        for g in range(1, GROUPS):
            acc = acc + per_core_outs[b * GROUPS + g]["outT"]
        out[b] = acc.T
    return out


def run(inputs, trace=False):
    nc = _get_program()
    in_maps = make_in_maps(inputs)
    res = run_bass_kernel_spmd(
        nc, in_maps, core_ids=list(range(NCORES)), trace=trace
    )
    return gather_output(res.results), res


def kernel(**inputs):
    out, _ = run(inputs, trace=False)
    return out


# revision 14
# speedup vs baseline: 2.6257x; 2.6257x over previous
"""Multi-head attention kernel for Trainium2, sharded over 8 NeuronCores.

Problem: B=2, S=2048, HIDDEN=1024, 16 heads, head_dim=64, fp32 in/out.

Sharding (data + tensor parallel per the hint): core c handles batch b=c//4
and head-group g=c%4 (4 heads = 256 hidden columns). QKV projections are
column-sharded, output projection row-sharded; each core returns a partial
out^T [1024, 2048] and the host sums the 4 partials per batch (the
row-parallel all-reduce) and transposes.

All matmul operands are bf16 (fp32 matmul runs at 1/4 PE rate: 2 half-speed
passes); accumulation is fp32 in PSUM and the final output is fp32. The
softmax denominator path (reciprocal) stays fp32.

Per-core device program (layouts chosen so no big intermediate needs a
transpose; only x is transposed once, on PE against a bf16 identity):
  x^T[k,s]   = PE transpose of x (cast to bf16 during the input DMA)
  Q^T/K^T    = Wq/Wk_cols^T @ x^T     -> [256, 2048] bf16, head-major
  V          = x @ Wv_cols            -> [2048, 256] natural, stored per
               kpos-tile with a ones column appended per head ([128, 4*65])
  scores^T   = K_h^T.T @ Q_h^T        -> [kpos 128, q 512] fp32 PSUM; the
               two heads of a 128-partition pair run row-packed (K=64,
               tile_position (0,0)/(64,0)) into two PSUM banks
  P^T        = exp(SCALE*scores^T + negmask[kpos])  (mask rides the ACT
               per-partition bias; scale is the free ACT affine) -> bf16
  ctx~^T     = [V_h | 1]^T @ P^T accumulated over kpos in PSUM -> [65, q];
               row 64 is the softmax denominator for free
  ctx^T     /= denom (fp32 reciprocal + PE broadcast via selector matmul)
  out^T     += Wo_rows^T @ ctx^T + bo_eff       (row-parallel partial, fp32)

bv is folded into bo_eff on the host (bv @ Wo_rows), bo added on one core
per batch group.
"""

import sys
import types

import numpy as np

import concourse.bass as bass
import concourse.tile as tile
from concourse import bacc, mybir
from concourse.bass_utils import run_bass_kernel_spmd
from concourse.masks import make_identity


def _install_ntff_hook_shim():
    """The agent image's antenv lacks axon_hooks, so trace=True dies on
    import. Recreate the module with the boot script's ctypes-based hook
    so NTFF profiling works."""
    if "antenv.axon_hooks" in sys.modules:
        return
    mod = types.ModuleType("antenv.axon_hooks")
    mod._hook = None

    def set_axon_ntff_profile_hook(h):
        mod._hook = h

    def get_axon_ntff_profile_hook():
        return mod._hook

    mod.set_axon_ntff_profile_hook = set_axon_ntff_profile_hook
    mod.get_axon_ntff_profile_hook = get_axon_ntff_profile_hook
    sys.modules["antenv.axon_hooks"] = mod
    try:
        from trn_agent_boot.trn_boot import _ntff_profile_via_ctypes

        mod._hook = _ntff_profile_via_ctypes("/opt/axon/libaxon_pjrt.so")
    except Exception:
        mod._hook = None


_install_ntff_hook_shim()

F32 = mybir.dt.float32
BF16 = mybir.dt.bfloat16

B = 2
S = 2048
HID = 1024
NH = 16  # total heads
DH = 64  # head dim
NCORES = 8
GROUPS = 4  # head groups (cores per batch)
NHC = 256  # hidden columns per core (4 heads * 64)
KT = 16  # kpos tiles of 128
SCALE = DH**-0.5

AF = mybir.ActivationFunctionType


def build_program():
    nc = bacc.Bacc(
        "TRN2",
        target_bir_lowering=False,
        debug=False,
        enable_asserts=False,
        num_devices=NCORES,
    )

    x_d = nc.dram_tensor("x", [S, HID], F32, kind="ExternalInput")
    wq_d = nc.dram_tensor("wq", [HID, NHC], F32, kind="ExternalInput")
    wk_d = nc.dram_tensor("wk", [HID, NHC], F32, kind="ExternalInput")
    wv_d = nc.dram_tensor("wv", [HID, NHC], F32, kind="ExternalInput")
    wo_d = nc.dram_tensor("wo", [NHC, HID], F32, kind="ExternalInput")
    bq_d = nc.dram_tensor("bq", [128, 2], F32, kind="ExternalInput")
    bk_d = nc.dram_tensor("bk", [128, 2], F32, kind="ExternalInput")
    bo_d = nc.dram_tensor("bo", [128, 8], F32, kind="ExternalInput")
    nm_d = nc.dram_tensor("negmask", [128, KT], F32, kind="ExternalInput")
    sel_d = nc.dram_tensor("sel", [128, 2, 128], F32, kind="ExternalInput")
    out_d = nc.dram_tensor("outT", [HID, S], F32, kind="ExternalOutput")

    with tile.TileContext(nc) as tc:
        with tc.tile_pool(name="persist", bufs=1) as persist:
            ident = persist.tile([128, 128], BF16, tag="ident")
            make_identity(nc, ident)
            negm = persist.tile([128, KT], F32, tag="negm")
            nc.sync.dma_start(out=negm, in_=nm_d[:])
            wo_sb = persist.tile([128, 2, HID], BF16, tag="wo")
            nc.gpsimd.dma_start(
                out=wo_sb, in_=wo_d[:].rearrange("(a p) n -> p a n", p=128)
            )
            bq_sb = persist.tile([128, 2], F32, tag="bq")
            nc.sync.dma_start(out=bq_sb, in_=bq_d[:])
            bk_sb = persist.tile([128, 2], F32, tag="bk")
            nc.sync.dma_start(out=bk_sb, in_=bk_d[:])
            bo_sb = persist.tile([128, 8], F32, tag="bo")
            nc.sync.dma_start(out=bo_sb, in_=bo_d[:])

            # Q^T / K^T head-pair tiles: partition = (head-in-pair, d),
            # free = (pair, q)
            qt = persist.tile([128, 2, S], BF16, tag="qt")
            kt_sb = persist.tile([128, 2, S], BF16, tag="kt")
            # V per kpos-tile, per head 64 cols + ones col: [128, KT, 4*65]
            vall = persist.tile([128, KT, 4 * 65], BF16, tag="vall")
            # ctx^T (same layout as qt), normalized in place later
            ctx_sb = persist.tile([128, 2, S], BF16, tag="ctx")
            # softmax denominators / reciprocals: head h lives on partition
            # 32*h (engine APs must start at a 32-partition boundary); unused
            # partitions are primed with 1.0 so the reciprocal stays finite.
            den = persist.tile([128, S], F32, tag="den")
            rec = persist.tile([128, S], F32, tag="rec")
            scr = persist.tile([128, S], F32, tag="scr")
            nc.vector.memset(den, 1.0)
            # selector for broadcasting recip rows across partitions
            sel = persist.tile([128, 2, 128], F32, tag="sel")
            nc.sync.dma_start(out=sel, in_=sel_d[:])

            # ---------------- Phase 1: x^T and QKV projections ----------------
            with tc.tile_pool(name="xnat", bufs=2) as xnat_p, \
                 tc.tile_pool(name="win", bufs=1) as win_p, \
                 tc.tile_pool(name="xtp", bufs=1) as xtp, \
                 tc.tile_pool(name="ps_t", bufs=2, space="PSUM") as ps_t, \
                 tc.tile_pool(name="ps_p", bufs=2, space="PSUM") as ps_p, \
                 tc.tile_pool(name="ps_v", bufs=2, space="PSUM") as ps_v:
                wq_sb = win_p.tile([128, 8, NHC], BF16, tag="wq")
                nc.gpsimd.dma_start(
                    out=wq_sb, in_=wq_d[:].rearrange("(a p) n -> p a n", p=128)
                )
                wk_sb = win_p.tile([128, 8, NHC], BF16, tag="wk")
                nc.gpsimd.dma_start(
                    out=wk_sb, in_=wk_d[:].rearrange("(a p) n -> p a n", p=128)
                )
                wv_sb = win_p.tile([128, 8, NHC], BF16, tag="wv")
                nc.gpsimd.dma_start(
                    out=wv_sb, in_=wv_d[:].rearrange("(a p) n -> p a n", p=128)
                )

                # x^T in SBUF: [k-in-tile 128, (ktile 8, s 2048)], bf16
                xT = xtp.tile([128, 8, S], BF16, tag="xT")
                x_re = x_d[:].rearrange("(c m p) k -> c p m k", p=128, m=2)
                for chunk in range(8):
                    # SWDGE cast-DMA: fp32 DRAM -> bf16 SBUF
                    xn = xnat_p.tile([128, 2, HID], BF16, tag="xn")
                    nc.gpsimd.dma_start(out=xn, in_=x_re[chunk])
                    for m in range(2):
                        s0 = chunk * 256 + m * 128
                        for kg in range(2):
                            tps = ps_t.tile([128, 4, 128], F32, tag="t")
                            for kk in range(4):
                                kj = kg * 4 + kk
                                nc.tensor.matmul(
                                    tps[:, kk, :],
                                    lhsT=xn[:, m, kj * 128 : (kj + 1) * 128],
                                    rhs=ident,
                                    start=True,
                                    stop=True,
                                )
                            nc.vector.tensor_copy(
                                xT[:, kg * 4 : (kg + 1) * 4, s0 : s0 + 128], tps
                            )

                # Q^T, K^T: lhsT = W cols, rhs = x^T
                for wsb, bsb, dst in ((wq_sb, bq_sb, qt), (wk_sb, bk_sb, kt_sb)):
                    for pt in range(2):
                        for j4 in range(4):
                            pp = ps_p.tile([128, 512], F32, tag="p")
                            for kj in range(8):
                                nc.tensor.matmul(
                                    pp,
                                    lhsT=wsb[:, kj, pt * 128 : (pt + 1) * 128],
                                    rhs=xT[:, kj, j4 * 512 : (j4 + 1) * 512],
                                    start=(kj == 0),
                                    stop=(kj == 7),
                                )
                            nc.vector.tensor_scalar_add(
                                dst[:, pt, j4 * 512 : (j4 + 1) * 512],
                                pp,
                                bsb[:, pt : pt + 1],
                            )

                # V natural: lhsT = x^T s-tiles, rhs = Wv
                for mt in range(KT):
                    pv = ps_v.tile([128, NHC], F32, tag="v")
                    for kj in range(8):
                        nc.tensor.matmul(
                            pv,
                            lhsT=xT[:, kj, mt * 128 : (mt + 1) * 128],
                            rhs=wv_sb[:, kj, :],
                            start=(kj == 0),
                            stop=(kj == 7),
                        )
                    v_slot = vall[:, mt, :].rearrange("p (h e) -> p h e", h=4)
                    nc.vector.tensor_copy(
                        v_slot[:, :, 0:64], pv.rearrange("p (h d) -> p h d", h=4)
                    )
                    nc.gpsimd.memset(v_slot[:, :, 64:65], 1.0)

            # ---------------- Phase 2: attention ----------------
            with tc.tile_pool(name="ps_s", bufs=2, space="PSUM") as ps_s, \
                 tc.tile_pool(name="ps_c", bufs=2, space="PSUM") as ps_c, \
                 tc.tile_pool(name="expp", bufs=3) as expp:
                for pt in range(2):
                    for j4 in range(4):
                        q0 = j4 * 512
                        cps = [
                            ps_c.tile([65, 512], F32, tag=f"c{h}", name=f"cps{h}")
                            for h in range(2)
                        ]
                        for t in range(KT):
                            sps = ps_s.tile([128, 2, 512], F32, tag="s")
                            for hh in range(2):
                                nc.tensor.matmul(
                                    sps[:, hh, :],
                                    lhsT=kt_sb[
                                        hh * 64 : (hh + 1) * 64,
                                        pt,
                                        t * 128 : (t + 1) * 128,
                                    ],
                                    rhs=qt[
                                        hh * 64 : (hh + 1) * 64, pt, q0 : q0 + 512
                                    ],
                                    start=True,
                                    stop=True,
                                    tile_position=(hh * 64, 0),
                                )
                            ep = expp.tile([128, 2, 512], BF16, tag="e")
                            nc.scalar.activation(
                                ep,
                                sps,
                                AF.Exp,
                                bias=negm[:, t : t + 1],
                                scale=float(SCALE),
                            )
                            for hh in range(2):
                                c0 = (2 * pt + hh) * 65
                                nc.tensor.matmul(
                                    cps[hh],
                                    lhsT=vall[:, t, c0 : c0 + 65],
                                    rhs=ep[:, hh, :],
                                    start=(t == 0),
                                    stop=(t == KT - 1),
                                )
                        for hh in range(2):
                            nc.vector.tensor_copy(
                                ctx_sb[hh * 64 : (hh + 1) * 64, pt, q0 : q0 + 512],
                                cps[hh][0:64, :],
                            )
                            h = 2 * pt + hh
                            nc.vector.tensor_copy(
                                den[32 * h : 32 * h + 1, q0 : q0 + 512],
                                cps[hh][64:65, :],
                            )

            # ---------------- Phase 3: normalize + output projection ----------
            nc.vector.reciprocal_approx_accurate(out=rec, in_=den, scratch=scr)
            with tc.tile_pool(name="ps_b", bufs=2, space="PSUM") as ps_b, \
                 tc.tile_pool(name="ps_o", bufs=2, space="PSUM") as ps_o, \
                 tc.tile_pool(name="bcp", bufs=2) as bcp, \
                 tc.tile_pool(name="outp", bufs=3) as outp:
                for pt in range(2):
                    for j4 in range(4):
                        q0 = j4 * 512
                        bc = ps_b.tile([128, 512], F32, tag="b")
                        nc.tensor.matmul(
                            bc,
                            lhsT=sel[:, pt, :],
                            rhs=rec[:, q0 : q0 + 512],
                            start=True,
                            stop=True,
                        )
                        bcs = bcp.tile([128, 512], BF16, tag="bc")
                        nc.vector.tensor_copy(bcs, bc)
                        nc.vector.tensor_mul(
                            ctx_sb[:, pt, q0 : q0 + 512],
                            ctx_sb[:, pt, q0 : q0 + 512],
                            bcs,
                        )
                out_re = out_d[:].rearrange("(a p) s -> a p s", p=128)
                for mt in range(8):
                    ot = outp.tile([128, S], F32, tag="o")
                    for j4 in range(4):
                        q0 = j4 * 512
                        po = ps_o.tile([128, 512], F32, tag="po")
                        for pt in range(2):
                            nc.tensor.matmul(
                                po,
                                lhsT=wo_sb[:, pt, mt * 128 : (mt + 1) * 128],
                                rhs=ctx_sb[:, pt, q0 : q0 + 512],
                                start=(pt == 0),
                                stop=(pt == 1),
                            )
                        # split PSUM->SBUF copies between DVE and ACT
                        if j4 % 2 == 0:
                            nc.vector.tensor_scalar_add(
                                ot[:, q0 : q0 + 512], po, bo_sb[:, mt : mt + 1]
                            )
                        else:
                            nc.scalar.add(
                                ot[:, q0 : q0 + 512], po, bo_sb[:, mt : mt + 1]
                            )
                    nc.sync.dma_start(out=out_re[mt], in_=ot)

    nc.compile()
    return nc


_PROGRAM = None


def _get_program():
    global _PROGRAM
    if _PROGRAM is None:
        _PROGRAM = build_program()
    return _PROGRAM


def make_in_maps(inputs):
    hs = np.asarray(inputs["hidden_states"], dtype=np.float32)
    mask = np.asarray(inputs["attention_mask"], dtype=np.float32)
    Wq = np.asarray(inputs["Wq"], dtype=np.float32)
    bq = np.asarray(inputs["bq"], dtype=np.float32)
    Wk = np.asarray(inputs["Wk"], dtype=np.float32)
    bk = np.asarray(inputs["bk"], dtype=np.float32)
    Wv = np.asarray(inputs["Wv"], dtype=np.float32)
    bv = np.asarray(inputs["bv"], dtype=np.float32)
    Wo = np.asarray(inputs["Wo"], dtype=np.float32)
    bo = np.asarray(inputs["bo"], dtype=np.float32)

    # selector: sel[k, pt, m] = 1 iff k == 32*(2*pt + m//64) (same on all
    # cores; head h's reciprocal lives on partition 32*h)
    sel = np.zeros((128, 2, 128), np.float32)
    for pt in range(2):
        for hh in range(2):
            sel[32 * (2 * pt + hh), pt, hh * 64 : (hh + 1) * 64] = 1.0

    in_maps = []
    for c in range(NCORES):
        b = c // GROUPS
        g = c % GROUPS
        cols = slice(g * NHC, (g + 1) * NHC)
        negm = ((1.0 - mask[b]) * -10000.0).astype(np.float32)
        # bv folded through the output projection; bo added on one core/batch
        bo_eff = bv[cols] @ Wo[cols, :]
        if g == 0:
            bo_eff = bo_eff + bo
        in_maps.append(
            {
                "x": np.ascontiguousarray(hs[b]),
                "wq": np.ascontiguousarray(Wq[:, cols]),
                "wk": np.ascontiguousarray(Wk[:, cols]),
                "wv": np.ascontiguousarray(Wv[:, cols]),
                "wo": np.ascontiguousarray(Wo[cols, :]),
                "bq": np.ascontiguousarray(bq[cols].reshape(2, 128).T),
                "bk": np.ascontiguousarray(bk[cols].reshape(2, 128).T),
                "bo": np.ascontiguousarray(
                    bo_eff.astype(np.float32).reshape(8, 128).T
                ),
                "negmask": np.ascontiguousarray(negm.reshape(KT, 128).T),
                "sel": sel,
            }
        )
    return in_maps


def gather_output(per_core_outs):
    out = np.empty((B, S, HID), dtype=np.float32)
    for b in range(B):
        acc = per_core_outs[b * GROUPS]["outT"].astype(np.float32)
        for g in range(1, GROUPS):
            acc = acc + per_core_outs[b * GROUPS + g]["outT"]
        out[b] = acc.T
    return out


def run(inputs, trace=False):
    nc = _get_program()
    in_maps = make_in_maps(inputs)
    res = run_bass_kernel_spmd(
        nc, in_maps, core_ids=list(range(NCORES)), trace=trace
    )
    return gather_output(res.results), res


def kernel(**inputs):
    out, _ = run(inputs, trace=False)
    return out
